# revision 3
# baseline (speedup 1.0000x reference)
"""Trainium2 Bass kernel for nn_BmmEnsemble (species-routed CELU-MLP ensemble).

Strategy (data-parallel over atoms, 8 NeuronCores):
  host: stable-sort atoms by species, shard each species block across the 8
        cores (capacity C atoms/species/core, zero-padded), pre-transpose aev
        to feature-major bf16, pre-pack weights.
  device (per core, SPMD): for each (species s, atom-tile t of T columns):
        L1: z1 = W1^T x           (bf16 matmuls, 10 M-chunks x 3 K-chunks)
        celu via the (r, u) split:  celu(z) = r + 0.1*u - 0.1,
              r = max(z+b, 0),  u = exp(10*min(z+b, 0))
        so L2/L3 contract against [r; u] with weights [W; 0.1W] and bias
        b_eff = b - 0.1*sum_fin(W) folded on host.
        L3 output is only needed as a per-feature SUM over atoms: ACT
        accum_out produces running sums of r3 and u3; no 4th matmul and no
        per-atom output.
  host finish: c3 = sum_t(acc_r + 0.1*acc_u) - 0.1*C, energy = W4 . c3 in
        f64, subtract zero-pad contributions, add b4 terms, divide by 8
        (ensemble mean).
"""
import numpy as np
import ml_dtypes

BF16 = ml_dtypes.bfloat16

S = 4            # species
M = 8            # ensemble models
F0, F1, F2, F3 = 384, 160, 128, 96
ALPHA = 0.1
N_CORES = 8
T = 400          # atoms per tile (matmul free dim; >=256 keeps f32r at 1 cyc/row)

_BUILD_CACHE = {}


# ----------------------------------------------------------------------------
# device kernel builder
# ----------------------------------------------------------------------------
def build_kernel(C):
    """Build (and cache) the compiled Bacc module for per-species-per-core
    capacity C (multiple of T)."""
    if C in _BUILD_CACHE:
        return _BUILD_CACHE[C]

    import concourse.bacc as bacc
    import concourse.tile as tile
    import concourse.mybir as mybir

    NT = C // T
    NUNIT = S * NT
    ACC_COLS = NUNIT * M
    F32 = mybir.dt.float32
    F32R = mybir.dt.float32r
    DBF = mybir.dt.bfloat16
    ADD, MAX, MIN = mybir.AluOpType.add, mybir.AluOpType.max, mybir.AluOpType.min
    RELU = mybir.ActivationFunctionType.Relu
    EXP = mybir.ActivationFunctionType.Exp

    nc = bacc.Bacc("TRN2", target_bir_lowering=False, debug=False)

    aev_d = nc.dram_tensor("aevT", [S, NT, 128, 3 * T], DBF, kind="ExternalInput").ap()
    w1_d = nc.dram_tensor("w1", [S, 128, 3 * 1280], DBF, kind="ExternalInput").ap()
    w2m_d = nc.dram_tensor("w2m", [S, 128, 2048], F32R, kind="ExternalInput").ap()
    w2r_d = nc.dram_tensor("w2r", [S, 128, 512], F32R, kind="ExternalInput").ap()
    w3_d = nc.dram_tensor("w3", [S, 128, 1536], F32R, kind="ExternalInput").ap()
    b1_d = nc.dram_tensor("b1c", [S, 128, 10], F32, kind="ExternalInput").ap()
    b2_d = nc.dram_tensor("b2c", [S, 128, 8], F32, kind="ExternalInput").ap()
    b3_d = nc.dram_tensor("b3c", [S, 128, 8], F32, kind="ExternalInput").ap()
    accr_d = nc.dram_tensor("accr", [128, ACC_COLS], F32, kind="ExternalOutput").ap()
    accu_d = nc.dram_tensor("accu", [128, ACC_COLS], F32, kind="ExternalOutput").ap()

    with tile.TileContext(nc) as tc:
        with tc.tile_pool(name="wpool", bufs=1) as wpool, \
             tc.tile_pool(name="xpool", bufs=2) as xpool, \
             tc.tile_pool(name="h1pool", bufs=1) as h1pool, \
             tc.tile_pool(name="h2pool", bufs=1) as h2pool, \
             tc.tile_pool(name="mpool", bufs=4) as mpool, \
             tc.tile_pool(name="s3pool", bufs=2) as s3pool, \
             tc.tile_pool(name="apool", bufs=1) as apool, \
             tc.tile_pool(name="ps", bufs=6, space="PSUM") as psp:

            # --- resident weights (all species) ---
            w1_t, w2m_t, w2r_t, w3_t, b1_t, b2_t, b3_t = [], [], [], [], [], [], []
            for s in range(S):
                w1_t.append(wpool.tile([128, 3 * 1280], DBF, tag=f"w1_{s}", name=f"w1_{s}"))
                nc.sync.dma_start(w1_t[s][:], w1_d[s])
                w2m_t.append(wpool.tile([128, 2048], F32R, tag=f"w2m_{s}", name=f"w2m_{s}"))
                nc.sync.dma_start(w2m_t[s][:], w2m_d[s])
                w2r_t.append(wpool.tile([128, 512], F32R, tag=f"w2r_{s}", name=f"w2r_{s}"))
                nc.sync.dma_start(w2r_t[s][:], w2r_d[s])
                w3_t.append(wpool.tile([128, 1536], F32R, tag=f"w3_{s}", name=f"w3_{s}"))
                nc.sync.dma_start(w3_t[s][:], w3_d[s])
                b1_t.append(wpool.tile([128, 10], F32, tag=f"b1_{s}", name=f"b1_{s}"))
                nc.sync.dma_start(b1_t[s][:], b1_d[s])
                b2_t.append(wpool.tile([128, 8], F32, tag=f"b2_{s}", name=f"b2_{s}"))
                nc.sync.dma_start(b2_t[s][:], b2_d[s])
                b3_t.append(wpool.tile([128, 8], F32, tag=f"b3_{s}", name=f"b3_{s}"))
                nc.sync.dma_start(b3_t[s][:], b3_d[s])

            acc_r = apool.tile([128, ACC_COLS], F32, tag="acc_r", name="acc_r")
            acc_u = apool.tile([128, ACC_COLS], F32, tag="acc_u", name="acc_u")
            nc.vector.memset(acc_r[:], 0.0)
            nc.vector.memset(acc_u[:], 0.0)

            # three elementwise ops per psum chunk; r-drain alternates ACT/DVE
            def drains(ps_ap, bias_ap, r_out, u_out, nrow, use_act_for_r,
                       racc=None, uacc=None):
                if use_act_for_r:
                    nc.scalar.activation(r_out, ps_ap, RELU, bias=bias_ap,
                                         scale=1.0,
                                         accum_out=racc)
                else:
                    assert racc is None
                    nc.vector.tensor_scalar(r_out, ps_ap, bias_ap, 0.0,
                                            op0=ADD, op1=MAX)
                m_t = mpool.tile([128, T], F32, tag="mscr", name="mscr")
                nc.vector.tensor_scalar(m_t[:nrow, :], ps_ap, bias_ap, 0.0,
                                        op0=ADD, op1=MIN)
                nc.scalar.activation(u_out, m_t[:nrow, :], EXP,
                                     bias=0.0, scale=10.0, accum_out=uacc)

            for s in range(S):
                for t in range(NT):
                    unit = s * NT + t
                    # stream this unit's aev chunk: [128, 3T] bf16
                    x_t = xpool.tile([128, 3 * T], DBF, tag="x", name="x_t")
                    nc.sync.dma_start(x_t[:], aev_d[s, t])

                    # ---- L1: 10 M-chunks (8 per-model mains + 2 rem regions)
                    r1 = h1pool.tile([128, 10 * T], F32R, tag="r1", name="r1")
                    u1 = h1pool.tile([128, 10 * T], F32R, tag="u1", name="u1")
                    for c in range(10):
                        ps = psp.tile([128, T], F32, tag="ps", name="ps", bufs=6)
                        for k in range(3):
                            nc.tensor.matmul(
                                ps[:],
                                w1_t[s][:, 1280 * k + 128 * c:1280 * k + 128 * (c + 1)],
                                x_t[:, k * T:(k + 1) * T],
                                start=(k == 0), stop=(k == 2))
                        drains(ps[:], b1_t[s][:, c:c + 1],
                               r1[:, c * T:(c + 1) * T], u1[:, c * T:(c + 1) * T],
                               128, use_act_for_r=(c % 2 == 0))

                    # ---- L2: per model; K = main(128) + rem(32), r and u halves
                    r2 = h2pool.tile([128, 8 * T], F32R, tag="r2", name="r2")
                    u2 = h2pool.tile([128, 8 * T], F32R, tag="u2", name="u2")
                    for m in range(M):
                        reg, j = m // 4, m % 4
                        ps = psp.tile([128, T], F32, tag="ps", name="ps", bufs=6)
                        nc.tensor.matmul(ps[:], w2m_t[s][:, (m * 2 + 0) * 128:(m * 2 + 1) * 128],
                                         r1[:, m * T:(m + 1) * T], start=True, stop=False)
                        nc.tensor.matmul(ps[:], w2m_t[s][:, (m * 2 + 1) * 128:(m * 2 + 2) * 128],
                                         u1[:, m * T:(m + 1) * T], start=False, stop=False)
                        nc.tensor.matmul(ps[:], w2r_t[s][32 * j:32 * (j + 1), (0 * 2 + reg) * 128:(0 * 2 + reg + 1) * 128],
                                         r1[32 * j:32 * (j + 1), (8 + reg) * T:(9 + reg) * T],
                                         start=False, stop=False, tile_position=(32 * j, 0))
                        nc.tensor.matmul(ps[:], w2r_t[s][32 * j:32 * (j + 1), (1 * 2 + reg) * 128:(1 * 2 + reg + 1) * 128],
                                         u1[32 * j:32 * (j + 1), (8 + reg) * T:(9 + reg) * T],
                                         start=False, stop=True, tile_position=(32 * j, 0))
                        drains(ps[:], b2_t[s][:, m:m + 1],
                               r2[:, m * T:(m + 1) * T], u2[:, m * T:(m + 1) * T],
                               128, use_act_for_r=(m % 2 == 0))

                    # ---- L3: per model, fout=96; only running sums survive
                    for m in range(M):
                        col = unit * M + m
                        ps = psp.tile([96, T], F32, tag="ps3", name="ps3", bufs=2)
                        nc.tensor.matmul(ps[:], w3_t[s][:, (m * 2 + 0) * 96:(m * 2 + 1) * 96],
                                         r2[:, m * T:(m + 1) * T], start=True, stop=False)
                        nc.tensor.matmul(ps[:], w3_t[s][:, (m * 2 + 1) * 96:(m * 2 + 2) * 96],
                                         u2[:, m * T:(m + 1) * T], start=False, stop=True)
                        r3 = s3pool.tile([96, T], F32, tag="r3scr", name="r3")
                        u3 = s3pool.tile([96, T], F32, tag="u3scr", name="u3")
                        drains(ps[:], b3_t[s][0:96, m:m + 1], r3[:], u3[:], 96,
                               use_act_for_r=True,
                               racc=acc_r[0:96, col:col + 1],
                               uacc=acc_u[0:96, col:col + 1])

            nc.sync.dma_start(accr_d, acc_r[:])
            nc.sync.dma_start(accu_d, acc_u[:])

    nc.compile()
    _BUILD_CACHE[C] = nc
    return nc


# ----------------------------------------------------------------------------
# host-side packing
# ----------------------------------------------------------------------------
def _celu64(x):
    return np.where(x > 0, x, ALPHA * np.expm1(np.minimum(x, 0) / ALPHA))


def prep_inputs(species, aev, W1, b1, W2, b2, W3, b3, W4, b4):
    """Returns (C, in_maps, finish) where finish(results) -> np scalar."""
    sp = np.asarray(species).reshape(-1)
    n_atoms = sp.shape[0]
    aev0 = np.asarray(aev, dtype=np.float32).reshape(n_atoms, F0)
    W1, b1, W2, b2, W3, b3, W4, b4 = [np.asarray(a, np.float64) for a in
                                      (W1, b1, W2, b2, W3, b3, W4, b4)]

    order = np.argsort(sp, kind="stable")
    cnt = np.bincount(sp.astype(np.int64), minlength=S)
    starts = np.concatenate([[0], np.cumsum(cnt)])
    C = max(T, int(-(-max(1, cnt.max()) // (N_CORES * T)) * T))
    NT = C // T

    # per-core feature-major aev: [S, NT, 128, 3T] bf16
    aevTs = [np.zeros((S, NT, 128, 3 * T), dtype=BF16) for _ in range(N_CORES)]
    for s in range(S):
        block = aev0[order[starts[s]:starts[s + 1]]]
        for c in range(N_CORES):
            seg = block[c * C:(c + 1) * C]
            if seg.shape[0] == 0:
                continue
            buf = np.zeros((C, F0), np.float32)
            buf[:seg.shape[0]] = seg
            # [C, 384] -> [NT, 128, 3T]:  [t, p, k*T+j] = buf[t*T+j, 128k+p]
            arr = buf.reshape(NT, T, 3, 128).transpose(0, 3, 2, 1).reshape(NT, 128, 3 * T)
            aevTs[c][s] = arr.astype(BF16)

    # ---- weights ----
    w1p = np.zeros((S, 128, 3 * 1280), dtype=BF16)
    b1c = np.zeros((S, 128, 10), np.float32)
    for s in range(S):
        cols = np.zeros((F0, 1280), np.float64)
        for c in range(8):
            cols[:, 128 * c:128 * (c + 1)] = W1[s, c, :, 0:128]
            b1c[s, :, c] = b1[s, c, 0, 0:128]
        for r in range(2):
            for j in range(4):
                cols[:, 1024 + 128 * r + 32 * j:1024 + 128 * r + 32 * (j + 1)] = \
                    W1[s, 4 * r + j, :, 128:160]
                b1c[s, 32 * j:32 * (j + 1), 8 + r] = b1[s, 4 * r + j, 0, 128:160]
        w1p[s] = cols.reshape(3, 128, 1280).transpose(1, 0, 2).reshape(128, 3840).astype(BF16)

    w2m = np.zeros((S, 128, 2048), np.float32)
    w2r = np.zeros((S, 128, 512), np.float32)
    b2c = np.zeros((S, 128, 8), np.float32)
    for s in range(S):
        for m in range(M):
            reg, j = m // 4, m % 4
            for ru, sc in ((0, 1.0), (1, ALPHA)):
                w2m[s, :, (m * 2 + ru) * 128:(m * 2 + ru + 1) * 128] = sc * W2[s, m, 0:128, :]
                w2r[s, 32 * j:32 * (j + 1), (ru * 2 + reg) * 128:(ru * 2 + reg + 1) * 128] = \
                    sc * W2[s, m, 128:160, :]
            b2c[s, :, m] = (b2[s, m, 0, :] - ALPHA * W2[s, m].sum(axis=0)).astype(np.float32)

    w3p = np.zeros((S, 128, 1536), np.float32)
    b3c = np.zeros((S, 128, 8), np.float32)
    for s in range(S):
        for m in range(M):
            for ru, sc in ((0, 1.0), (1, ALPHA)):
                w3p[s, :, (m * 2 + ru) * 96:(m * 2 + ru + 1) * 96] = sc * W3[s, m, :, :]
            b3c[s, 0:96, m] = (b3[s, m, 0, :] - ALPHA * W3[s, m].sum(axis=0)).astype(np.float32)

    common = {"w1": w1p[..., :], "w2m": w2m, "w2r": w2r, "w3": w3p,
              "b1c": b1c, "b2c": b2c, "b3c": b3c}
    in_maps = [dict(common, aevT=aevTs[c]) for c in range(N_CORES)]

    # ---- host finishing constants (f64) ----
    # zero-input chain value per species (pad correction), without b4
    e_pad = np.zeros(S)
    for s in range(S):
        h = _celu64(b1[s, :, 0, :])                       # [M, 160]
        h = _celu64(np.einsum("mf,mfo->mo", h, W2[s]) + b2[s, :, 0, :])
        h = _celu64(np.einsum("mf,mfo->mo", h, W3[s]) + b3[s, :, 0, :])
        e_pad[s] = np.einsum("mf,mf->", h, W4[s, :, :, 0])
    b4sum = b4[:, :, 0, 0].sum(axis=1)                    # [S]

    def finish(results):
        accr = np.zeros((128, S * NT * M), np.float64)
        accu = np.zeros_like(accr)
        for res in results:
            accr += res["accr"].astype(np.float64)
            accu += res["accu"].astype(np.float64)
        # c3[s, m, f] = sum over all N_CORES*C padded atoms of celu(z3)
        tot = 0.0
        for s in range(S):
            c3 = np.zeros((M, F3))
            for t in range(NT):
                cols = (s * NT + t) * M
                c3 += (accr[0:96, cols:cols + M] + ALPHA * accu[0:96, cols:cols + M]).T
            c3 -= ALPHA * C * N_CORES
            tot += np.einsum("mf,mf->", c3, W4[s, :, :, 0])
            n_pad = N_CORES * C - cnt[s]
            tot -= n_pad * e_pad[s]
            tot += cnt[s] * b4sum[s]
        return np.array([tot / M], dtype=np.float32)

    return C, in_maps, finish


def kernel(**inputs):
    from concourse.bass_utils import run_bass_kernel_spmd
    C, in_maps, finish = prep_inputs(**inputs)
    nc = build_kernel(C)
    res = run_bass_kernel_spmd(nc, in_maps, list(range(N_CORES)))
    return finish(res.results)


# revision 9
# speedup vs baseline: 1.4608x; 1.4608x over previous
"""Trainium2 Bass kernel for nn_BmmEnsemble (species-routed CELU-MLP ensemble).

Strategy (data-parallel over atoms, 8 NeuronCores):
  host: stable-sort atoms by species, shard each species block across the 8
        cores (capacity C atoms/species/core, zero-padded), pre-transpose aev
        to feature-major bf16, pre-pack weights.
  device (per core, SPMD): for each (species s, atom-tile t of T columns):
        L1: z1 = W1^T x           (bf16 matmuls, 10 M-chunks x 3 K-chunks)
        celu via the (r, u) split:  celu(z) = r + 0.1*u - 0.1,
              r = max(z+b, 0),  u = exp(10*min(z+b, 0))
        so L2/L3 contract against [r; u] with weights [W; 0.1W] and bias
        b_eff = b - 0.1*sum_fin(W) folded on host.
        L3 output is only needed as a per-feature SUM over atoms: ACT
        accum_out produces running sums of r3 and u3; no 4th matmul and no
        per-atom output.
  host finish: c3 = sum_t(acc_r + 0.1*acc_u) - 0.1*C, energy = W4 . c3 in
        f64, subtract zero-pad contributions, add b4 terms, divide by 8
        (ensemble mean).
"""
import numpy as np
import ml_dtypes

BF16 = ml_dtypes.bfloat16

S = 4            # species
M = 8            # ensemble models
F0, F1, F2, F3 = 384, 160, 128, 96
ALPHA = 0.1
N_CORES = 8
T = 400          # atoms per tile (matmul free dim; >=256 keeps f32r at 1 cyc/row)

_BUILD_CACHE = {}


# ----------------------------------------------------------------------------
# device kernel builder
# ----------------------------------------------------------------------------
def build_kernel(C):
    """Build (and cache) the compiled Bacc module for per-species-per-core
    capacity C (multiple of T)."""
    if C in _BUILD_CACHE:
        return _BUILD_CACHE[C]

    import concourse.bacc as bacc
    import concourse.tile as tile
    import concourse.mybir as mybir

    NT = C // T
    NUNIT = S * NT
    ACC_COLS = NUNIT * M
    F32 = mybir.dt.float32
    F32R = mybir.dt.float32r
    DBF = mybir.dt.bfloat16
    ADD, MAX, MIN = mybir.AluOpType.add, mybir.AluOpType.max, mybir.AluOpType.min
    RELU = mybir.ActivationFunctionType.Relu
    EXP = mybir.ActivationFunctionType.Exp
    BANK = 512  # f32 elements per PSUM bank

    nc = bacc.Bacc("TRN2", target_bir_lowering=False, debug=False)

    aev_d = nc.dram_tensor("aevT", [S, NT, 128, 3 * T], DBF, kind="ExternalInput").ap()
    w1_d = nc.dram_tensor("w1", [S, 128, 3 * 1280], DBF, kind="ExternalInput").ap()
    w2m_d = nc.dram_tensor("w2m", [S, 128, 2048], F32R, kind="ExternalInput").ap()
    w2r_d = nc.dram_tensor("w2r", [S, 128, 512], F32R, kind="ExternalInput").ap()
    w3_d = nc.dram_tensor("w3", [S, 128, 1536], F32R, kind="ExternalInput").ap()
    bl1_d = nc.dram_tensor("bl1", [S, 128, 384], F32R, kind="ExternalInput").ap()
    bl2_d = nc.dram_tensor("bl2", [S, 128, 256], F32R, kind="ExternalInput").ap()
    bl3_d = nc.dram_tensor("bl3", [S, 128, 192], F32R, kind="ExternalInput").ap()
    ones_d = nc.dram_tensor("ones", [128, T], F32R, kind="ExternalInput").ap()
    accr_d = nc.dram_tensor("accr", [128, ACC_COLS], F32, kind="ExternalOutput").ap()
    accu_d = nc.dram_tensor("accu", [128, ACC_COLS], F32, kind="ExternalOutput").ap()

    with tile.TileContext(nc) as tc:
        with tc.tile_pool(name="wpool", bufs=1) as wpool, \
             tc.tile_pool(name="w1pool", bufs=2) as w1pool, \
             tc.tile_pool(name="xpool", bufs=2) as xpool, \
             tc.tile_pool(name="h1pool", bufs=1) as h1pool, \
             tc.tile_pool(name="h2pool", bufs=1) as h2pool, \
             tc.tile_pool(name="upool", bufs=4) as upool, \
             tc.tile_pool(name="s3pool", bufs=2) as s3pool, \
             tc.tile_pool(name="apool", bufs=1) as apool, \
             tc.tile_pool(name="ps", bufs=2, space="PSUM") as psp:

            # --- per-species weights, DMA'd just-in-time inside the loop ---
            w2m_t, w2r_t, w3_t, bl1_t, bl2_t, bl3_t = [], [], [], [], [], []
            for s in range(S):
                w2m_t.append(wpool.tile([128, 2048], F32R, tag=f"w2m_{s}", name=f"w2m_{s}"))
                w2r_t.append(wpool.tile([128, 512], F32R, tag=f"w2r_{s}", name=f"w2r_{s}"))
                w3_t.append(wpool.tile([128, 1536], F32R, tag=f"w3_{s}", name=f"w3_{s}"))
                bl1_t.append(wpool.tile([128, 384], F32R, tag=f"bl1_{s}", name=f"bl1_{s}"))
                bl2_t.append(wpool.tile([128, 256], F32R, tag=f"bl2_{s}", name=f"bl2_{s}"))
                bl3_t.append(wpool.tile([128, 192], F32R, tag=f"bl3_{s}", name=f"bl3_{s}"))

            ones_t = wpool.tile([128, T], F32R, tag="ones", name="ones")
            nc.sync.dma_start(ones_t[:], ones_d)

            acc_r = apool.tile([128, ACC_COLS], F32, tag="acc_r", name="acc_r")
            acc_u = apool.tile([128, ACC_COLS], F32, tag="acc_u", name="acc_u")
            nc.vector.memset(acc_r[:], 0.0)
            nc.vector.memset(acc_u[:], 0.0)

            def group_view(ps_t, nrow, gsz):
                # strided view [nrow, gsz, T] over bank-aligned chunks
                return ps_t[0:nrow, :].rearrange("p (g q) -> p g q", q=BANK)[:, 0:gsz, 0:T]

            def bias_mms(ps_t, bl_t, nrow, gsz, c0, mwid):
                # chunk c0+g gets bias row at partition 32*((c0+g)%4), col block (c0+g)//4
                for g in range(gsz):
                    c = c0 + g
                    blk, j = c // 4, c % 4
                    nc.tensor.matmul(
                        ps_t[0:nrow, g * BANK:g * BANK + T],
                        bl_t[32 * j:32 * j + 1, blk * mwid:blk * mwid + nrow],
                        ones_t[32 * j:32 * j + 1, 0:T],
                        start=False, stop=True, tile_position=(32 * j, 0))

            # batched drains over one psum group (chunks are bias-complete)
            def drains(ps_t, nrow, gsz, r_view, u_view, r_on_act):
                ps_v = group_view(ps_t, nrow, gsz)
                ur = upool.tile([128, 2 * T], F32, tag="uscr", name="uscr")
                ur_v = ur[0:nrow, 0:gsz * T].rearrange("p (g q) -> p g q", q=T)
                nc.scalar.activation(ur_v, ps_v, EXP, bias=0.0, scale=10.0)
                nc.gpsimd.tensor_scalar(u_view, ur_v, 1.0, None, op0=MIN)
                if r_on_act:
                    nc.scalar.activation(r_view, ps_v, RELU, bias=0.0, scale=1.0)
                else:
                    nc.vector.tensor_scalar(r_view, ps_v, 0.0, None, op0=MAX)
                return ur

            for s in range(S):
                w1s = w1pool.tile([128, 3 * 1280], DBF, tag="w1", name=f"w1s_{s}")
                nc.sync.dma_start(w1s[:], w1_d[s])
                nc.sync.dma_start(bl1_t[s][:], bl1_d[s])
                x0_t = xpool.tile([128, 3 * T], DBF, tag="x", name="x_t")
                nc.sync.dma_start(x0_t[:], aev_d[s, 0])
                nc.sync.dma_start(w2m_t[s][:], w2m_d[s])
                nc.sync.dma_start(w2r_t[s][:], w2r_d[s])
                nc.sync.dma_start(bl2_t[s][:], bl2_d[s])
                nc.sync.dma_start(w3_t[s][:], w3_d[s])
                nc.sync.dma_start(bl3_t[s][:], bl3_d[s])
                for t in range(NT):
                    unit = s * NT + t
                    if t == 0:
                        x_t = x0_t
                    else:
                        x_t = xpool.tile([128, 3 * T], DBF, tag="x", name="x_t")
                        nc.sync.dma_start(x_t[:], aev_d[s, t])

                    # ---- L1: 10 M-chunks in groups of [4, 4, 2]
                    r1 = h1pool.tile([128, 10 * T], F32R, tag="r1", name="r1")
                    u1 = h1pool.tile([128, 10 * T], F32R, tag="u1", name="u1")
                    for c0, gsz in ((0, 2), (2, 2), (4, 2), (6, 2), (8, 2)):
                        ps_t = psp.tile([128, 2 * BANK], F32, tag="psg", name="psg", bufs=4)
                        for g in range(gsz):
                            c = c0 + g
                            for k in range(3):
                                nc.tensor.matmul(
                                    ps_t[:, g * BANK:g * BANK + T],
                                    w1s[:, 1280 * k + 128 * c:1280 * k + 128 * (c + 1)],
                                    x_t[:, k * T:(k + 1) * T],
                                    start=(k == 0), stop=False)
                        bias_mms(ps_t, bl1_t[s], 128, gsz, c0, 128)
                        rv = r1[:, c0 * T:(c0 + gsz) * T].rearrange("p (g q) -> p g q", q=T)
                        uv = u1[:, c0 * T:(c0 + gsz) * T].rearrange("p (g q) -> p g q", q=T)
                        drains(ps_t, 128, gsz, rv, uv, r_on_act=False)

                    # ---- L2: 8 models in 2 groups of 4 (group == rem region)
                    r2 = h2pool.tile([128, 8 * T], F32R, tag="r2", name="r2")
                    u2 = h2pool.tile([128, 8 * T], F32R, tag="u2", name="u2")
                    for half in range(4):
                        m0 = 2 * half
                        reg = m0 // 4
                        ps_t = psp.tile([128, 2 * BANK], F32, tag="psg", name="psg", bufs=4)
                        for g in range(2):
                            m = m0 + g
                            sl = slice(g * BANK, g * BANK + T)
                            nc.tensor.matmul(ps_t[:, sl],
                                             w2m_t[s][:, (m * 2 + 0) * 128:(m * 2 + 1) * 128],
                                             r1[:, m * T:(m + 1) * T], start=True, stop=False)
                            nc.tensor.matmul(ps_t[:, sl],
                                             w2m_t[s][:, (m * 2 + 1) * 128:(m * 2 + 2) * 128],
                                             u1[:, m * T:(m + 1) * T], start=False, stop=False)
                            for ru in range(2):
                                h = (r1, u1)[ru]
                                j = m % 4
                                nc.tensor.matmul(
                                    ps_t[:, sl],
                                    w2r_t[s][32 * j:32 * (j + 1), (ru * 2 + reg) * 128:(ru * 2 + reg + 1) * 128],
                                    h[32 * j:32 * (j + 1), (8 + reg) * T:(9 + reg) * T],
                                    start=False, stop=False, tile_position=(32 * j, 0))
                        bias_mms(ps_t, bl2_t[s], 128, 2, m0, 128)
                        rv = r2[:, m0 * T:(m0 + 2) * T].rearrange("p (g q) -> p g q", q=T)
                        uv = u2[:, m0 * T:(m0 + 2) * T].rearrange("p (g q) -> p g q", q=T)
                        drains(ps_t, 128, 2, rv, uv, r_on_act=False)

                    # ---- L3: 8 models in 2 groups of 4; only sums survive
                    for grp in range(4):
                        ps_t = psp.tile([96, 2 * BANK], F32, tag="psg", name="psg3", bufs=4)
                        for g in range(2):
                            m = 2 * grp + g
                            sl = slice(g * BANK, g * BANK + T)
                            nc.tensor.matmul(ps_t[:, sl],
                                             w3_t[s][:, (m * 2 + 0) * 96:(m * 2 + 1) * 96],
                                             r2[:, m * T:(m + 1) * T], start=True, stop=False)
                            nc.tensor.matmul(ps_t[:, sl],
                                             w3_t[s][:, (m * 2 + 1) * 96:(m * 2 + 2) * 96],
                                             u2[:, m * T:(m + 1) * T], start=False, stop=False)
                        bias_mms(ps_t, bl3_t[s], 96, 2, 2 * grp, 96)
                        # batched exp, per-chunk min/relu with running-sum accums
                        ps_v = group_view(ps_t, 96, 2)
                        ur = upool.tile([128, 2 * T], F32, tag="uscr", name="uscr3")
                        ur_v = ur[0:96, 0:2 * T].rearrange("p (g q) -> p g q", q=T)
                        nc.scalar.activation(ur_v, ps_v, EXP, bias=0.0, scale=10.0)
                        for g in range(2):
                            m = 2 * grp + g
                            col = unit * M + m
                            r3 = s3pool.tile([96, T], F32, tag="r3scr", name="r3")
                            u3 = s3pool.tile([96, T], F32, tag="u3scr", name="u3")
                            if m % 2 == 0:
                                nc.scalar.activation(
                                    r3[:], ps_t[0:96, g * BANK:g * BANK + T],
                                    RELU, bias=0.0, scale=1.0,
                                    accum_out=acc_r[0:96, col:col + 1])
                            else:
                                nc.vector.tensor_scalar(
                                    r3[:], ps_t[0:96, g * BANK:g * BANK + T],
                                    0.0, None, op0=MAX, op1=ADD,
                                    accum_out=acc_r[0:96, col:col + 1])
                            nc.vector.tensor_scalar(
                                u3[:], ur[0:96, g * T:(g + 1) * T], 1.0, None,
                                op0=MIN, op1=ADD,
                                accum_out=acc_u[0:96, col:col + 1])

            nc.sync.dma_start(accr_d, acc_r[:])
            nc.sync.dma_start(accu_d, acc_u[:])

    nc.compile()
    _BUILD_CACHE[C] = nc
    return nc


# ----------------------------------------------------------------------------
# host-side packing
# ----------------------------------------------------------------------------
def _celu64(x):
    return np.where(x > 0, x, ALPHA * np.expm1(np.minimum(x, 0) / ALPHA))


def prep_inputs(species, aev, W1, b1, W2, b2, W3, b3, W4, b4):
    """Returns (C, in_maps, finish) where finish(results) -> np scalar."""
    sp = np.asarray(species).reshape(-1)
    n_atoms = sp.shape[0]
    aev0 = np.asarray(aev, dtype=np.float32).reshape(n_atoms, F0)
    W1, b1, W2, b2, W3, b3, W4, b4 = [np.asarray(a, np.float64) for a in
                                      (W1, b1, W2, b2, W3, b3, W4, b4)]

    order = np.argsort(sp, kind="stable")
    cnt = np.bincount(sp.astype(np.int64), minlength=S)
    starts = np.concatenate([[0], np.cumsum(cnt)])
    C = max(T, int(-(-max(1, cnt.max()) // (N_CORES * T)) * T))
    NT = C // T

    # per-core feature-major aev: [S, NT, 128, 3T] bf16
    aevTs = [np.zeros((S, NT, 128, 3 * T), dtype=BF16) for _ in range(N_CORES)]
    for s in range(S):
        block = aev0[order[starts[s]:starts[s + 1]]]
        for c in range(N_CORES):
            seg = block[c * C:(c + 1) * C]
            if seg.shape[0] == 0:
                continue
            buf = np.zeros((C, F0), np.float32)
            buf[:seg.shape[0]] = seg
            # [C, 384] -> [NT, 128, 3T]:  [t, p, k*T+j] = buf[t*T+j, 128k+p]
            arr = buf.reshape(NT, T, 3, 128).transpose(0, 3, 2, 1).reshape(NT, 128, 3 * T)
            aevTs[c][s] = arr.astype(BF16)

    # ---- weights ----
    w1p = np.zeros((S, 128, 3 * 1280), dtype=BF16)
    b1c = np.zeros((S, 128, 10), np.float32)   # per-chunk bias columns
    for s in range(S):
        cols = np.zeros((F0, 1280), np.float64)
        for c in range(8):
            cols[:, 128 * c:128 * (c + 1)] = W1[s, c, :, 0:128]
            b1c[s, :, c] = b1[s, c, 0, 0:128]
        for r in range(2):
            for j in range(4):
                cols[:, 1024 + 128 * r + 32 * j:1024 + 128 * r + 32 * (j + 1)] = \
                    W1[s, 4 * r + j, :, 128:160]
                b1c[s, 32 * j:32 * (j + 1), 8 + r] = b1[s, 4 * r + j, 0, 128:160]
        w1p[s] = cols.reshape(3, 128, 1280).transpose(1, 0, 2).reshape(128, 3840).astype(BF16)

    w2m = np.zeros((S, 128, 2048), np.float32)
    w2r = np.zeros((S, 128, 512), np.float32)
    b2c = np.zeros((S, 128, 8), np.float32)
    for s in range(S):
        for m in range(M):
            reg, j = m // 4, m % 4
            for ru, sc in ((0, 1.0), (1, ALPHA)):
                w2m[s, :, (m * 2 + ru) * 128:(m * 2 + ru + 1) * 128] = sc * W2[s, m, 0:128, :]
                w2r[s, 32 * j:32 * (j + 1), (ru * 2 + reg) * 128:(ru * 2 + reg + 1) * 128] = \
                    sc * W2[s, m, 128:160, :]
            b2c[s, :, m] = (b2[s, m, 0, :] - ALPHA * W2[s, m].sum(axis=0)).astype(np.float32)

    w3p = np.zeros((S, 128, 1536), np.float32)
    b3c = np.zeros((S, 128, 8), np.float32)
    for s in range(S):
        for m in range(M):
            for ru, sc in ((0, 1.0), (1, ALPHA)):
                w3p[s, :, (m * 2 + ru) * 96:(m * 2 + ru + 1) * 96] = sc * W3[s, m, :, :]
            b3c[s, 0:96, m] = (b3[s, m, 0, :] - ALPHA * W3[s, m].sum(axis=0)).astype(np.float32)

    # bias-row lhsT tensors: chunk c -> partition 32*(c%4), col block c//4
    bl1 = np.zeros((S, 128, 384), np.float32)
    for s in range(S):
        for c in range(10):
            bl1[s, 32 * (c % 4), (c // 4) * 128:(c // 4) * 128 + 128] = b1c[s, :, c]
    bl2 = np.zeros((S, 128, 256), np.float32)
    for s in range(S):
        for c in range(8):
            bl2[s, 32 * (c % 4), (c // 4) * 128:(c // 4) * 128 + 128] = b2c[s, :, c]
    bl3 = np.zeros((S, 128, 192), np.float32)
    for s in range(S):
        for c in range(8):
            bl3[s, 32 * (c % 4), (c // 4) * 96:(c // 4) * 96 + 96] = b3c[s, 0:96, c]

    common = {"w1": w1p, "w2m": w2m, "w2r": w2r, "w3": w3p,
              "bl1": bl1, "bl2": bl2, "bl3": bl3,
              "ones": np.ones((128, T), np.float32)}
    in_maps = [dict(common, aevT=aevTs[c]) for c in range(N_CORES)]

    # ---- host finishing constants (f64) ----
    # zero-input chain value per species (pad correction), without b4
    e_pad = np.zeros(S)
    for s in range(S):
        h = _celu64(b1[s, :, 0, :])                       # [M, 160]
        h = _celu64(np.einsum("mf,mfo->mo", h, W2[s]) + b2[s, :, 0, :])
        h = _celu64(np.einsum("mf,mfo->mo", h, W3[s]) + b3[s, :, 0, :])
        e_pad[s] = np.einsum("mf,mf->", h, W4[s, :, :, 0])
    b4sum = b4[:, :, 0, 0].sum(axis=1)                    # [S]

    def finish(results):
        accr = np.zeros((128, S * NT * M), np.float64)
        accu = np.zeros_like(accr)
        for res in results:
            accr += res["accr"].astype(np.float64)
            accu += res["accu"].astype(np.float64)
        # c3[s, m, f] = sum over all N_CORES*C padded atoms of celu(z3)
        tot = 0.0
        for s in range(S):
            c3 = np.zeros((M, F3))
            for t in range(NT):
                cols = (s * NT + t) * M
                c3 += (accr[0:96, cols:cols + M] + ALPHA * accu[0:96, cols:cols + M]).T
            c3 -= ALPHA * C * N_CORES
            tot += np.einsum("mf,mf->", c3, W4[s, :, :, 0])
            n_pad = N_CORES * C - cnt[s]
            tot -= n_pad * e_pad[s]
            tot += cnt[s] * b4sum[s]
        return np.array([tot / M], dtype=np.float32)

    return C, in_maps, finish


def kernel(**inputs):
    from concourse.bass_utils import run_bass_kernel_spmd
    C, in_maps, finish = prep_inputs(**inputs)
    nc = build_kernel(C)
    res = run_bass_kernel_spmd(nc, in_maps, list(range(N_CORES)))
    return finish(res.results)


# revision 13
# speedup vs baseline: 1.5423x; 1.0558x over previous
"""Trainium2 Bass kernel for nn_BmmEnsemble (species-routed CELU-MLP ensemble).

Strategy (data-parallel over atoms, 8 NeuronCores):
  host: stable-sort atoms by species, shard each species block across the 8
        cores (capacity C atoms/species/core, zero-padded), pre-transpose aev
        to feature-major bf16, pre-pack weights.
  device (per core, SPMD): for each (species s, atom-tile t of T columns):
        L1: z1 = W1^T x           (bf16 matmuls, 10 M-chunks x 3 K-chunks)
        celu via the (r, u) split:  celu(z) = r + 0.1*u - 0.1,
              r = max(z+b, 0),  u = exp(10*min(z+b, 0))
        so L2/L3 contract against [r; u] with weights [W; 0.1W] and bias
        b_eff = b - 0.1*sum_fin(W) folded on host.
        L3 output is only needed as a per-feature SUM over atoms: ACT
        accum_out produces running sums of r3 and u3; no 4th matmul and no
        per-atom output.
  host finish: c3 = sum_t(acc_r + 0.1*acc_u) - 0.1*C, energy = W4 . c3 in
        f64, subtract zero-pad contributions, add b4 terms, divide by 8
        (ensemble mean).
"""
import numpy as np
import ml_dtypes

BF16 = ml_dtypes.bfloat16

S = 4            # species
M = 8            # ensemble models
F0, F1, F2, F3 = 384, 160, 128, 96
ALPHA = 0.1
N_CORES = 8
T = 400          # atoms per tile (matmul free dim; >=256 keeps f32r at 1 cyc/row)

_BUILD_CACHE = {}


# ----------------------------------------------------------------------------
# device kernel builder
# ----------------------------------------------------------------------------
def build_kernel(C):
    """Build (and cache) the compiled Bacc module for per-species-per-core
    capacity C (multiple of T)."""
    if C in _BUILD_CACHE:
        return _BUILD_CACHE[C]

    import concourse.bacc as bacc
    import concourse.tile as tile
    import concourse.mybir as mybir

    NT = C // T
    NUNIT = S * NT
    ACC_COLS = NUNIT * M
    F32 = mybir.dt.float32
    F32R = mybir.dt.float32r
    DBF = mybir.dt.bfloat16
    ADD, MAX, MIN = mybir.AluOpType.add, mybir.AluOpType.max, mybir.AluOpType.min
    RELU = mybir.ActivationFunctionType.Relu
    EXP = mybir.ActivationFunctionType.Exp
    BANK = 512  # f32 elements per PSUM bank

    nc = bacc.Bacc("TRN2", target_bir_lowering=False, debug=False)

    aev_d = nc.dram_tensor("aevT", [S, NT, 128, 3 * T], DBF, kind="ExternalInput").ap()
    w1_d = nc.dram_tensor("w1", [S, 128, 3 * 1280], DBF, kind="ExternalInput").ap()
    w2m_d = nc.dram_tensor("w2m", [S, 128, 2048], F32R, kind="ExternalInput").ap()
    w2r_d = nc.dram_tensor("w2r", [S, 128, 512], F32R, kind="ExternalInput").ap()
    w3_d = nc.dram_tensor("w3", [S, 128, 1536], F32R, kind="ExternalInput").ap()
    bl1_d = nc.dram_tensor("bl1", [S, 128, 384], F32R, kind="ExternalInput").ap()
    bl2_d = nc.dram_tensor("bl2", [S, 128, 256], F32R, kind="ExternalInput").ap()
    bl3_d = nc.dram_tensor("bl3", [S, 128, 192], F32R, kind="ExternalInput").ap()
    ones_d = nc.dram_tensor("ones", [128, T], F32R, kind="ExternalInput").ap()
    accr_d = nc.dram_tensor("accr", [128, ACC_COLS], F32, kind="ExternalOutput").ap()
    accu_d = nc.dram_tensor("accu", [128, ACC_COLS], F32, kind="ExternalOutput").ap()

    with tile.TileContext(nc) as tc:
        with tc.tile_pool(name="wpool", bufs=1) as wpool, \
             tc.tile_pool(name="w1pool", bufs=2) as w1pool, \
             tc.tile_pool(name="xpool", bufs=2) as xpool, \
             tc.tile_pool(name="h1pool", bufs=1) as h1pool, \
             tc.tile_pool(name="h2pool", bufs=1) as h2pool, \
             tc.tile_pool(name="upool", bufs=4) as upool, \
             tc.tile_pool(name="s3pool", bufs=2) as s3pool, \
             tc.tile_pool(name="apool", bufs=1) as apool, \
             tc.tile_pool(name="ps", bufs=2, space="PSUM") as psp:

            # --- per-species weights, DMA'd just-in-time inside the loop ---
            w2m_t, w2r_t, w3_t, bl1_t, bl2_t, bl3_t = [], [], [], [], [], []
            for s in range(S):
                w2m_t.append(wpool.tile([128, 2048], F32R, tag=f"w2m_{s}", name=f"w2m_{s}"))
                w2r_t.append(wpool.tile([128, 512], F32R, tag=f"w2r_{s}", name=f"w2r_{s}"))
                w3_t.append(wpool.tile([128, 1536], F32R, tag=f"w3_{s}", name=f"w3_{s}"))
                bl1_t.append(wpool.tile([128, 384], F32R, tag=f"bl1_{s}", name=f"bl1_{s}"))
                bl2_t.append(wpool.tile([128, 256], F32R, tag=f"bl2_{s}", name=f"bl2_{s}"))
                bl3_t.append(wpool.tile([128, 192], F32R, tag=f"bl3_{s}", name=f"bl3_{s}"))

            ones_t = wpool.tile([128, T], F32R, tag="ones", name="ones")
            nc.sync.dma_start(ones_t[:], ones_d)

            acc_r = apool.tile([128, ACC_COLS], F32, tag="acc_r", name="acc_r")
            acc_u = apool.tile([128, ACC_COLS], F32, tag="acc_u", name="acc_u")
            nc.vector.memset(acc_r[:], 0.0)
            nc.vector.memset(acc_u[:], 0.0)

            def group_view(ps_t, nrow, gsz):
                # strided view [nrow, gsz, T] over bank-aligned chunks
                return ps_t[0:nrow, :].rearrange("p (g q) -> p g q", q=BANK)[:, 0:gsz, 0:T]

            def bias_mms(ps_t, bl_t, nrow, gsz, c0, mwid):
                # chunk c0+g gets bias row at partition 32*((c0+g)%4), col block (c0+g)//4
                for g in range(gsz):
                    c = c0 + g
                    blk, j = c // 4, c % 4
                    nc.tensor.matmul(
                        ps_t[0:nrow, g * BANK:g * BANK + T],
                        bl_t[32 * j:32 * j + 1, blk * mwid:blk * mwid + nrow],
                        ones_t[32 * j:32 * j + 1, 0:T],
                        start=False, stop=True, tile_position=(32 * j, 0))

            # batched drains over one psum group (chunks are bias-complete)
            def drains(ps_t, nrow, gsz, r_view, u_view, r_on_act):
                ps_v = group_view(ps_t, nrow, gsz)
                ur = upool.tile([128, 2 * T], F32, tag="uscr", name="uscr")
                ur_v = ur[0:nrow, 0:gsz * T].rearrange("p (g q) -> p g q", q=T)
                nc.scalar.activation(ur_v, ps_v, EXP, bias=0.0, scale=10.0)
                nc.gpsimd.tensor_scalar(u_view, ur_v, 1.0, None, op0=MIN)
                if r_on_act:
                    nc.scalar.activation(r_view, ps_v, RELU, bias=0.0, scale=1.0)
                else:
                    nc.vector.tensor_scalar(r_view, ps_v, 0.0, None, op0=MAX)
                return ur

            for s in range(S):
                w1s = w1pool.tile([128, 3 * 1280], DBF, tag="w1", name=f"w1s_{s}")
                for k in range(3):
                    nc.sync.dma_start(w1s[:, 1280 * k:1280 * (k + 1)],
                                      w1_d[s][:, 1280 * k:1280 * (k + 1)])
                nc.sync.dma_start(bl1_t[s][:], bl1_d[s])
                x0_t = xpool.tile([128, 3 * T], DBF, tag="x", name="x_t")
                nc.sync.dma_start(x0_t[:], aev_d[s, 0])
                nc.sync.dma_start(w2m_t[s][:], w2m_d[s])
                nc.sync.dma_start(w2r_t[s][:], w2r_d[s])
                nc.sync.dma_start(bl2_t[s][:], bl2_d[s])
                nc.sync.dma_start(w3_t[s][:], w3_d[s])
                nc.sync.dma_start(bl3_t[s][:], bl3_d[s])
                for t in range(NT):
                    unit = s * NT + t
                    if t == 0:
                        x_t = x0_t
                    else:
                        x_t = xpool.tile([128, 3 * T], DBF, tag="x", name="x_t")
                        nc.sync.dma_start(x_t[:], aev_d[s, t])

                    # ---- L1: 10 M-chunks in groups of [4, 4, 2]
                    r1 = h1pool.tile([128, 10 * T], F32R, tag="r1", name="r1")
                    u1 = h1pool.tile([128, 10 * T], F32R, tag="u1", name="u1")
                    for c0, gsz in ((0, 2), (2, 2), (4, 2), (6, 2), (8, 2)):
                        ps_t = psp.tile([128, 2 * BANK], F32, tag="psg", name="psg", bufs=4)
                        for g in range(gsz):
                            c = c0 + g
                            for k in range(3):
                                nc.tensor.matmul(
                                    ps_t[:, g * BANK:g * BANK + T],
                                    w1s[:, 1280 * k + 128 * c:1280 * k + 128 * (c + 1)],
                                    x_t[:, k * T:(k + 1) * T],
                                    start=(k == 0), stop=False)
                        bias_mms(ps_t, bl1_t[s], 128, gsz, c0, 128)
                        rv = r1[:, c0 * T:(c0 + gsz) * T].rearrange("p (g q) -> p g q", q=T)
                        uv = u1[:, c0 * T:(c0 + gsz) * T].rearrange("p (g q) -> p g q", q=T)
                        drains(ps_t, 128, gsz, rv, uv, r_on_act=(c0 == 0))

                    # ---- L2: 8 models in 2 groups of 4 (group == rem region)
                    r2 = h2pool.tile([128, 8 * T], F32R, tag="r2", name="r2")
                    u2 = h2pool.tile([128, 8 * T], F32R, tag="u2", name="u2")
                    for half in range(4):
                        m0 = 2 * half
                        reg = m0 // 4
                        ps_t = psp.tile([128, 2 * BANK], F32, tag="psg", name="psg", bufs=4)
                        for g in range(2):
                            m = m0 + g
                            sl = slice(g * BANK, g * BANK + T)
                            nc.tensor.matmul(ps_t[:, sl],
                                             w2m_t[s][:, (m * 2 + 0) * 128:(m * 2 + 1) * 128],
                                             r1[:, m * T:(m + 1) * T], start=True, stop=False)
                            nc.tensor.matmul(ps_t[:, sl],
                                             w2m_t[s][:, (m * 2 + 1) * 128:(m * 2 + 2) * 128],
                                             u1[:, m * T:(m + 1) * T], start=False, stop=False)
                            for ru in range(2):
                                h = (r1, u1)[ru]
                                j = m % 4
                                nc.tensor.matmul(
                                    ps_t[:, sl],
                                    w2r_t[s][32 * j:32 * (j + 1), (ru * 2 + reg) * 128:(ru * 2 + reg + 1) * 128],
                                    h[32 * j:32 * (j + 1), (8 + reg) * T:(9 + reg) * T],
                                    start=False, stop=False, tile_position=(32 * j, 0))
                        bias_mms(ps_t, bl2_t[s], 128, 2, m0, 128)
                        rv = r2[:, m0 * T:(m0 + 2) * T].rearrange("p (g q) -> p g q", q=T)
                        uv = u2[:, m0 * T:(m0 + 2) * T].rearrange("p (g q) -> p g q", q=T)
                        drains(ps_t, 128, 2, rv, uv, r_on_act=(half == 0))

                    # ---- L3: 8 models in 2 groups of 4; only sums survive
                    for grp in range(4):
                        ps_t = psp.tile([96, 2 * BANK], F32, tag="psg", name="psg3", bufs=4)
                        for g in range(2):
                            m = 2 * grp + g
                            sl = slice(g * BANK, g * BANK + T)
                            nc.tensor.matmul(ps_t[:, sl],
                                             w3_t[s][:, (m * 2 + 0) * 96:(m * 2 + 1) * 96],
                                             r2[:, m * T:(m + 1) * T], start=True, stop=False)
                            nc.tensor.matmul(ps_t[:, sl],
                                             w3_t[s][:, (m * 2 + 1) * 96:(m * 2 + 2) * 96],
                                             u2[:, m * T:(m + 1) * T], start=False, stop=False)
                        bias_mms(ps_t, bl3_t[s], 96, 2, 2 * grp, 96)
                        # batched exp, per-chunk min/relu with running-sum accums
                        ps_v = group_view(ps_t, 96, 2)
                        ur = upool.tile([128, 2 * T], F32, tag="uscr", name="uscr3")
                        ur_v = ur[0:96, 0:2 * T].rearrange("p (g q) -> p g q", q=T)
                        nc.scalar.activation(ur_v, ps_v, EXP, bias=0.0, scale=10.0)
                        for g in range(2):
                            m = 2 * grp + g
                            col = unit * M + m
                            r3 = s3pool.tile([96, T], F32, tag="r3scr", name="r3")
                            u3 = s3pool.tile([96, T], F32, tag="u3scr", name="u3")
                            nc.vector.tensor_scalar(
                                r3[:], ps_t[0:96, g * BANK:g * BANK + T],
                                0.0, None, op0=MAX, op1=ADD,
                                accum_out=acc_r[0:96, col:col + 1])
                            nc.vector.tensor_scalar(
                                u3[:], ur[0:96, g * T:(g + 1) * T], 1.0, None,
                                op0=MIN, op1=ADD,
                                accum_out=acc_u[0:96, col:col + 1])

            nc.sync.dma_start(accr_d, acc_r[:])
            nc.sync.dma_start(accu_d, acc_u[:])

    nc.compile()
    _BUILD_CACHE[C] = nc
    return nc


# ----------------------------------------------------------------------------
# host-side packing
# ----------------------------------------------------------------------------
def _celu64(x):
    return np.where(x > 0, x, ALPHA * np.expm1(np.minimum(x, 0) / ALPHA))


def prep_inputs(species, aev, W1, b1, W2, b2, W3, b3, W4, b4):
    """Returns (C, in_maps, finish) where finish(results) -> np scalar."""
    sp = np.asarray(species).reshape(-1)
    n_atoms = sp.shape[0]
    aev0 = np.asarray(aev, dtype=np.float32).reshape(n_atoms, F0)
    W1, b1, W2, b2, W3, b3, W4, b4 = [np.asarray(a, np.float64) for a in
                                      (W1, b1, W2, b2, W3, b3, W4, b4)]

    order = np.argsort(sp, kind="stable")
    cnt = np.bincount(sp.astype(np.int64), minlength=S)
    starts = np.concatenate([[0], np.cumsum(cnt)])
    C = max(T, int(-(-max(1, cnt.max()) // (N_CORES * T)) * T))
    NT = C // T

    # per-core feature-major aev: [S, NT, 128, 3T] bf16
    aevTs = [np.zeros((S, NT, 128, 3 * T), dtype=BF16) for _ in range(N_CORES)]
    for s in range(S):
        block = aev0[order[starts[s]:starts[s + 1]]]
        for c in range(N_CORES):
            seg = block[c * C:(c + 1) * C]
            if seg.shape[0] == 0:
                continue
            buf = np.zeros((C, F0), np.float32)
            buf[:seg.shape[0]] = seg
            # [C, 384] -> [NT, 128, 3T]:  [t, p, k*T+j] = buf[t*T+j, 128k+p]
            arr = buf.reshape(NT, T, 3, 128).transpose(0, 3, 2, 1).reshape(NT, 128, 3 * T)
            aevTs[c][s] = arr.astype(BF16)

    # ---- weights ----
    w1p = np.zeros((S, 128, 3 * 1280), dtype=BF16)
    b1c = np.zeros((S, 128, 10), np.float32)   # per-chunk bias columns
    for s in range(S):
        cols = np.zeros((F0, 1280), np.float64)
        for c in range(8):
            cols[:, 128 * c:128 * (c + 1)] = W1[s, c, :, 0:128]
            b1c[s, :, c] = b1[s, c, 0, 0:128]
        for r in range(2):
            for j in range(4):
                cols[:, 1024 + 128 * r + 32 * j:1024 + 128 * r + 32 * (j + 1)] = \
                    W1[s, 4 * r + j, :, 128:160]
                b1c[s, 32 * j:32 * (j + 1), 8 + r] = b1[s, 4 * r + j, 0, 128:160]
        w1p[s] = cols.reshape(3, 128, 1280).transpose(1, 0, 2).reshape(128, 3840).astype(BF16)

    w2m = np.zeros((S, 128, 2048), np.float32)
    w2r = np.zeros((S, 128, 512), np.float32)
    b2c = np.zeros((S, 128, 8), np.float32)
    for s in range(S):
        for m in range(M):
            reg, j = m // 4, m % 4
            for ru, sc in ((0, 1.0), (1, ALPHA)):
                w2m[s, :, (m * 2 + ru) * 128:(m * 2 + ru + 1) * 128] = sc * W2[s, m, 0:128, :]
                w2r[s, 32 * j:32 * (j + 1), (ru * 2 + reg) * 128:(ru * 2 + reg + 1) * 128] = \
                    sc * W2[s, m, 128:160, :]
            b2c[s, :, m] = (b2[s, m, 0, :] - ALPHA * W2[s, m].sum(axis=0)).astype(np.float32)

    w3p = np.zeros((S, 128, 1536), np.float32)
    b3c = np.zeros((S, 128, 8), np.float32)
    for s in range(S):
        for m in range(M):
            for ru, sc in ((0, 1.0), (1, ALPHA)):
                w3p[s, :, (m * 2 + ru) * 96:(m * 2 + ru + 1) * 96] = sc * W3[s, m, :, :]
            b3c[s, 0:96, m] = (b3[s, m, 0, :] - ALPHA * W3[s, m].sum(axis=0)).astype(np.float32)

    # bias-row lhsT tensors: chunk c -> partition 32*(c%4), col block c//4
    bl1 = np.zeros((S, 128, 384), np.float32)
    for s in range(S):
        for c in range(10):
            bl1[s, 32 * (c % 4), (c // 4) * 128:(c // 4) * 128 + 128] = b1c[s, :, c]
    bl2 = np.zeros((S, 128, 256), np.float32)
    for s in range(S):
        for c in range(8):
            bl2[s, 32 * (c % 4), (c // 4) * 128:(c // 4) * 128 + 128] = b2c[s, :, c]
    bl3 = np.zeros((S, 128, 192), np.float32)
    for s in range(S):
        for c in range(8):
            bl3[s, 32 * (c % 4), (c // 4) * 96:(c // 4) * 96 + 96] = b3c[s, 0:96, c]

    common = {"w1": w1p, "w2m": w2m, "w2r": w2r, "w3": w3p,
              "bl1": bl1, "bl2": bl2, "bl3": bl3,
              "ones": np.ones((128, T), np.float32)}
    in_maps = [dict(common, aevT=aevTs[c]) for c in range(N_CORES)]

    # ---- host finishing constants (f64) ----
    # zero-input chain value per species (pad correction), without b4
    e_pad = np.zeros(S)
    for s in range(S):
        h = _celu64(b1[s, :, 0, :])                       # [M, 160]
        h = _celu64(np.einsum("mf,mfo->mo", h, W2[s]) + b2[s, :, 0, :])
        h = _celu64(np.einsum("mf,mfo->mo", h, W3[s]) + b3[s, :, 0, :])
        e_pad[s] = np.einsum("mf,mf->", h, W4[s, :, :, 0])
    b4sum = b4[:, :, 0, 0].sum(axis=1)                    # [S]

    def finish(results):
        accr = np.zeros((128, S * NT * M), np.float64)
        accu = np.zeros_like(accr)
        for res in results:
            accr += res["accr"].astype(np.float64)
            accu += res["accu"].astype(np.float64)
        # c3[s, m, f] = sum over all N_CORES*C padded atoms of celu(z3)
        tot = 0.0
        for s in range(S):
            c3 = np.zeros((M, F3))
            for t in range(NT):
                cols = (s * NT + t) * M
                c3 += (accr[0:96, cols:cols + M] + ALPHA * accu[0:96, cols:cols + M]).T
            c3 -= ALPHA * C * N_CORES
            tot += np.einsum("mf,mf->", c3, W4[s, :, :, 0])
            n_pad = N_CORES * C - cnt[s]
            tot -= n_pad * e_pad[s]
            tot += cnt[s] * b4sum[s]
        return np.array([tot / M], dtype=np.float32)

    return C, in_maps, finish


def kernel(**inputs):
    from concourse.bass_utils import run_bass_kernel_spmd
    C, in_maps, finish = prep_inputs(**inputs)
    nc = build_kernel(C)
    res = run_bass_kernel_spmd(nc, in_maps, list(range(N_CORES)))
    return finish(res.results)
